# revision 17
# baseline (speedup 1.0000x reference)
"""Trainium2 Bass kernel for nn_CgpHmmCell: HMM forward-algorithm log-likelihood.

Math: A is banded-cyclic (row i nonzero only at cols (i+k)%S, k=0..3), so
a@A is a 4-tap stencil. Emissions come from one-hot inputs, i.e. emi = B[:, obs].

Device algorithm (per core, 4 of the 32 sequences):
  - stream one-hot x [4,T,101], extract obs via iota-dot (DVE accumulate)
  - build per-step gather indices; Pool ap_gather pulls emission tiles from a
    replicated B-table
  - serial scan: 5 banded-block matmuls (bf16, PE, PSUM-accumulated; tiny K=3
    matmuls couple neighboring blocks incl. the cyclic wrap) then one DVE
    multiply by the emission tile
  - renormalize every R steps; scale folded into a later emission tile
    (off the critical path); loglik accumulated via ACT Ln
State layout: 5 blocks x 125 states on partitions 0..124, cols = block*4+seq.
"""
import numpy as np
import ml_dtypes

import concourse.bass as bass
import concourse.mybir as mybir
import concourse.tile as tile
from concourse import library_config
from concourse.bass_utils import run_bass_kernel_spmd
from concourse.vector_clock import ScopedClock
from concourse import bass2jax

# ---------------------------------------------------------------- tile patch
# This walrus build rejects >1 sync wait on one CTRL instruction; Tile's exit
# drain gathers one wait per outstanding proc semaphore onto a single Drain.
# Split the waits onto a chain of drains.


def _drain_and_barrier_split(self, tick_clock, wait_clock):
    nc = self.nc
    drain_inst = nc.sync.drain()
    wait_clock.add_sem_waits(
        drain_inst.ins, ScopedClock({None: tick_clock.global_clock})
    )
    si = drain_inst.ins.sync_info
    if si is not None and si.on_wait and len(si.on_wait) > 1:
        waits = list(si.on_wait)
        del si.on_wait[:]
        si.on_wait.extend(waits[:1])
        for w in waits[1:]:
            nop = nc.sync.drain()
            nsi = nop.ins.sync_info
            if nsi is None:
                nop.ins.sync_info = type(si)(on_wait=[w], on_update=[])
            else:
                nsi.on_wait.append(w)
    nc.all_engine_barrier()
    assert self.sems is not None
    popped = nc._tile_sem_poison_stack.pop()
    assert popped is self._sem_poison
    nc.clear_and_free_semaphores(list(self.sems.allocated().values()))
    nc.all_engine_barrier()


tile.TileContext._drain_and_barrier = _drain_and_barrier_split


def _split_multi_waits(nc, max_waits=1):
    """Walrus here allows only one sync wait per instruction; move extra waits
    onto same-engine EventSemaphore carriers inserted just before."""
    for fn in nc.m.functions:
        for bb in fn.blocks:
            new = []
            for inst in bb.instructions:
                si = inst.sync_info
                if si is not None and si.on_wait and len(si.on_wait) > max_waits:
                    waits = list(si.on_wait)
                    del si.on_wait[:]
                    si.on_wait.extend(waits[:max_waits])
                    extra = waits[max_waits:]
                    for j in range(0, len(extra), max_waits):
                        es = mybir.InstEventSemaphore(
                            name=f"wsplit-{nc.next_id()}", engine=inst.engine
                        )
                        es.sync_info = mybir.SyncInfo(
                            on_wait=extra[j:j + max_waits], on_update=[]
                        )
                        new.append(es)
                new.append(inst)
            bb.instructions[:] = new

# ---------------------------------------------------------------- constants
S = 612
EE = 101          # emission columns incl. terminal
NCTX, ALPH = 25, 4
NB, NET = 5, 125  # state blocks x net states per block
NSEQ = 4          # sequences per core
NCORES = 8
T_FULL = 8192
R = 8             # renormalize every R steps
LAG = 3           # scale applied into emission LAG steps after measurement

f32 = mybir.dt.float32
bf16 = mybir.dt.bfloat16
u16 = mybir.dt.uint16
AO = mybir.AluOpType


def _host_params(init_kernel, transition_kernel, emission_kernel):
    """Build A-band coefficients, emission table, init vector (fp32 numpy)."""
    tk = transition_kernel.astype(np.float32)
    ek = emission_kernel.astype(np.float32)
    # A row i: logits [w_{3i}, w_{3i+1}, w_{3i+2}, 1.0] at cols i..i+3 (mod S)
    logits = np.stack([tk[0::3], tk[1::3], tk[2::3], np.ones(S, np.float32)], axis=1)
    m = logits.max(-1, keepdims=True)
    ex = np.exp(logits - m)
    Arow = ex / ex.sum(-1, keepdims=True)          # Arow[i,k] = A[i,(i+k)%S]
    # ck[k, j] = A[(j-k)%S, j]
    j = np.arange(S)
    ck = np.zeros((4, NB * NET), np.float32)
    for k in range(4):
        ck[k, j] = Arow[(j - k) % S, k]
    # emission probs: grouped softmax over (state, context) letter groups
    lg = ek.reshape(S - 1, NCTX - 1, ALPH)
    full = np.concatenate([np.ones((S - 1, 1, ALPH), np.float32), lg], axis=1)
    mx = full.max(-1, keepdims=True)
    e2 = np.exp(full - mx)
    probs = (e2 / e2.sum(-1, keepdims=True)).reshape(S - 1, NCTX * ALPH)
    Bfull = np.zeros((S, EE), np.float32)
    Bfull[: S - 1, : EE - 1] = probs
    Bfull[S - 1, EE - 1] = 1.0
    ikf = init_kernel.astype(np.float32)
    Iv = np.exp(ikf - ikf.max())
    Iv /= Iv.sum()

    # weight tiles: Wm [128, NB*128] bf16 : Wm[:, 128c+p] col = out state 125c+p
    Wm = np.zeros((128, NB * 128), np.float32)
    for c in range(NB):
        for p in range(NET):
            st = NET * c + p
            if st >= S:
                continue
            for k in range(4):
                r = p - k
                if 0 <= r < NET:
                    Wm[r, 128 * c + p] = ck[k, st]
        # cols 125..127 duplicate this block's last 3 real states so the next
        # step's boundary matmul can read them from partition base 0
        last3 = 122 if c < NB - 1 else 109
        for i in range(3):
            Wm[:, 128 * c + 125 + i] = Wm[:, 128 * c + last3 + i]
    # boundary: Wb [32, NB*3]: contraction over the 32-partition boundary tile
    # (rows 29..31 = prev block's last-3 states); out p (<= i)
    Wb = np.zeros((32, NB * 3), np.float32)
    for c in range(NB):
        for i in range(3):
            for p in range(i + 1):
                st = NET * c + p
                if st < S:
                    Wb[29 + i, 3 * c + p] = ck[p + 3 - i, st]
    # B gather table [128, NB*EE]; rows 125..127 carry the emission values of
    # each block's last-3 real states (for the boundary tile's e-multiply)
    Btab = np.zeros((128, NB * EE), np.float32)
    for c in range(NB):
        lo, hi = NET * c, min(NET * (c + 1), S)
        Btab[: hi - lo, EE * c: EE * c + EE] = Bfull[lo:hi]
        last3 = NET * c + (122 if c < NB - 1 else 109)
        Btab[125:128, EE * c: EE * c + EE] = Bfull[last3:last3 + 3]
    # init vector tile [128, 20]
    Ivt = np.zeros((128, NB * NSEQ), np.float32)
    for b in range(NSEQ):
        Ivt[:4, b] = Iv          # block c=0 cols 0..3, states 0..3
    # iota [128, EE]
    Iota = np.tile(np.arange(EE, dtype=np.float32), (128, 1))
    # idx base [128, 2]: idx j = (p%16) + 16*col ; c = j//4 ; junk lanes -> 0
    Base = np.zeros((128, 2), np.float32)
    for p in range(128):
        Base[p, 0] = ((p % 16) // 4) * EE
        Base[p, 1] = 4 * EE if (p % 16) < 4 else 0.0
    # ones has zeros at rows 125..127 so the duplicated boundary rows don't
    # double-count in the partition-sum Z reduction
    ones = np.ones((128, 1), np.float32)
    ones[125:] = 0.0
    onesrow = np.ones((1, 128), np.float32)
    ident = np.eye(128, dtype=np.float32)
    return dict(
        wm=Wm.astype(ml_dtypes.bfloat16),
        wb=Wb.astype(ml_dtypes.bfloat16),
        btab=Btab,
        ivt=Ivt,
        iota=Iota,
        base=Base,
        ones=ones,
        onesrow=onesrow,
        ident=ident,
    )


def _build_program(T):
    assert T % 128 == 0
    ntiles = NSEQ * T // 128
    nhalf = (ntiles + 127) // 128
    TH = T // 128                      # X-tiles (and transposed rows) per seq

    nc = bass.Bass()
    x_d = nc.dram_tensor("x", [NSEQ * T, EE], f32, kind="ExternalInput")
    wm_d = nc.dram_tensor("wm", [128, NB * 128], bf16, kind="ExternalInput")
    wb_d = nc.dram_tensor("wb", [32, NB * 3], bf16, kind="ExternalInput")
    btab_d = nc.dram_tensor("btab", [128, NB * EE], f32, kind="ExternalInput")
    ivt_d = nc.dram_tensor("ivt", [128, NB * NSEQ], f32, kind="ExternalInput")
    iota_d = nc.dram_tensor("iota", [128, EE], f32, kind="ExternalInput")
    base_d = nc.dram_tensor("base", [128, 2], f32, kind="ExternalInput")
    ones_d = nc.dram_tensor("ones", [128, 1], f32, kind="ExternalInput")
    onesrow_d = nc.dram_tensor("onesrow", [1, 128], f32, kind="ExternalInput")
    ident_d = nc.dram_tensor("ident", [128, 128], f32, kind="ExternalInput")
    obsf_d = nc.dram_tensor("obsf", [NSEQ, T], f32)        # internal bounce
    ll_d = nc.dram_tensor("ll", [1, NSEQ], f32, kind="ExternalOutput")

    NC20 = NB * NSEQ   # 20

    with tile.TileContext(nc) as tc:
        with (
            tc.tile_pool(name="const", bufs=1) as cpool,
            tc.tile_pool(name="xs", bufs=4) as xpool,
            tc.tile_pool(name="scr", bufs=2) as spool,
            tc.tile_pool(name="es", bufs=6) as epool,
            tc.tile_pool(name="ps", bufs=4, space="PSUM") as ppool,
            tc.tile_pool(name="zps", bufs=1, space="PSUM") as zpool,
            tc.tile_pool(name="bps", bufs=1, space="PSUM") as bpool,
            tc.tile_pool(name="tps", bufs=1, space="PSUM") as tpool,
        ):
            sbW = cpool.tile([128, NB * 128], bf16)
            sbWb = cpool.tile([32, NB * 3], bf16)
            sbB = cpool.tile([128, NB * EE], f32)
            sbI = cpool.tile([128, NC20], f32)
            sbIota = cpool.tile([128, EE], f32)
            sbBase = cpool.tile([128, 2], f32)
            sbOnes = cpool.tile([128, 1], f32)
            sbOnesRow = cpool.tile([1, 128], f32)
            sbId = cpool.tile([128, 128], f32)
            for sb, d in ((sbW, wm_d), (sbWb, wb_d), (sbB, btab_d), (sbI, ivt_d),
                          (sbIota, iota_d), (sbBase, base_d), (sbOnes, ones_d),
                          (sbOnesRow, onesrow_d), (sbId, ident_d)):
                nc.sync.dma_start(out=sb[:], in_=d[:])

            ObsNat = cpool.tile([128, ntiles], f32)
            Trs = cpool.tile([128, nhalf, 128], f32)
            o_rep = cpool.tile([128, T], f32)
            IdxAll = cpool.tile([128, 2 * T], u16)
            a0 = cpool.tile([128, NC20], bf16)
            a1 = cpool.tile([128, NC20], bf16)
            a_tiles = [a0, a1]
            bnd0 = cpool.tile([32, NC20], bf16)
            bnd1 = cpool.tile([32, NC20], bf16)
            bnd_tiles = [bnd0, bnd1]
            llacc = cpool.tile([1, NSEQ], f32)
            lnz = cpool.tile([1, NSEQ], f32)
            zinv = cpool.tile([1, NSEQ], f32)
            zinv20 = cpool.tile([1, NC20], f32)
            zbro_ref = {}

            # ---- phase A: stream one-hot, extract obs ----
            for k in range(ntiles):
                xt = xpool.tile([128, EE], f32, tag="xt")
                nc.sync.dma_start(out=xt[:], in_=x_d[128 * k:128 * (k + 1), :])
                scr = spool.tile([128, EE], f32, tag="scr")
                nc.vector.scalar_tensor_tensor(
                    out=scr[:], in0=xt[:], scalar=1.0, in1=sbIota[:],
                    op0=AO.mult, op1=AO.mult,
                    accum_out=ObsNat[:, k:k + 1],
                )
            # transpose ObsNat -> Trs (obs values laid t-minor)
            for h in range(nhalf):
                w = min(128, ntiles - 128 * h)
                tp = tpool.tile([128, 128], f32, tag="tp")
                nc.tensor.transpose(tp[0:w, :], ObsNat[:, 128 * h:128 * h + w],
                                    sbId[:])
                nc.scalar.activation(out=Trs[0:w, h, :], in_=tp[0:w, :],
                                     func=mybir.ActivationFunctionType.Copy)
            # Trs rows kk = b*TH + thi hold obs[b, thi*128 + tlo]; store per seq
            for b in range(NSEQ):
                kk0, kk1 = b * TH, (b + 1) * TH
                h0, h1 = kk0 // 128, (kk1 - 1) // 128
                if h0 == h1:
                    src = Trs[kk0 - 128 * h0: kk1 - 128 * h0, h0, :]
                    nc.sync.dma_start(out=obsf_d[b, :], in_=src)
                else:
                    mid = 128 * h1
                    nc.sync.dma_start(out=obsf_d[b, 0:(mid - kk0) * 128],
                                      in_=Trs[kk0 - 128 * h0:128, h0, :])
                    nc.sync.dma_start(out=obsf_d[b, (mid - kk0) * 128:T],
                                      in_=Trs[0: kk1 - mid, h1, :])
            # replicate to o_rep[p, t] = obs[p%4, t]
            for m in range(32):
                nc.sync.dma_start(out=o_rep[4 * m:4 * m + 4, :], in_=obsf_d[:, :])
            # IdxAll[p, 2t+col] = Base[p,col] + obs[p%4, t]
            for col in range(2):
                nc.vector.tensor_scalar(
                    out=IdxAll[:, col: 2 * T: 2], in0=o_rep[:, :],
                    scalar1=sbBase[:, col:col + 1], scalar2=None, op0=AO.add,
                )

            # ---- helpers ----
            sbW_c = [sbW[:, 128 * c:128 * (c + 1)] for c in range(NB)]
            sbWb_c = [sbWb[:, 3 * c:3 * (c + 1)] for c in range(NB)]

            def gather_E(t):
                E = epool.tile([128, NC20], f32, tag="E")
                nc.gpsimd.indirect_copy(E[:], sbB[:], IdxAll[:, 2 * t:2 * t + 2],
                                        True)
                return E

            def renorm(t, a_cur):
                """Measure Z, accumulate loglik, prep 1/Z broadcast tile."""
                Zb = spool.tile([128, NSEQ], f32, tag="Zb")
                nc.vector.tensor_reduce(
                    out=Zb[:], in_=a_cur[:].rearrange("p (c b) -> p b c", b=NSEQ),
                    axis=mybir.AxisListType.X, op=AO.add,
                )
                zr = zpool.tile([1, NSEQ], f32, tag="zr")
                nc.tensor.matmul(zr[:], sbOnes[:], Zb[:], start=True, stop=True)
                nc.scalar.activation(out=lnz[:], in_=zr[:],
                                     func=mybir.ActivationFunctionType.Ln)
                if t == 0:
                    nc.vector.tensor_copy(llacc[:], lnz[:])
                else:
                    nc.vector.tensor_add(llacc[:], llacc[:], lnz[:])
                if t != T - 1:
                    nc.vector.reciprocal(zinv[:], zr[:])
                    nc.vector.tensor_copy(
                        zinv20[:].rearrange("p (c b) -> p c b", b=NSEQ),
                        zinv[:].unsqueeze(1).broadcast_to([1, NB, NSEQ]),
                    )
                    zb = bpool.tile([128, NC20], f32, tag="zbro")
                    nc.tensor.matmul(zb[:], sbOnesRow[:], zinv20[:],
                                     start=True, stop=True)
                    zbro_ref[0] = zb

            # ---- t = 0 ----
            E0 = gather_E(0)
            nc.vector.tensor_tensor(out=a0[:], in0=E0[:], in1=sbI[:],
                                    op=AO.mult)
            # initial alpha is nonzero only at states 0..3, so all boundary
            # states are zero
            nc.vector.memset(bnd0[:], 0.0)
            renorm(0, a0)
            scale_steps = {LAG}

            # ---- scan ----
            for t in range(1, T):
                E = gather_E(t)
                Eap = E[:]
                if t in scale_steps:
                    Esc = epool.tile([128, NC20], f32, tag="Esc")
                    nc.vector.tensor_tensor(out=Esc[:], in0=Eap,
                                            in1=zbro_ref[0][:], op=AO.mult)
                    Eap = Esc[:]
                a_cur = a_tiles[(t - 1) % 2]
                a_nxt = a_tiles[t % 2]
                b_cur = bnd_tiles[(t - 1) % 2]
                b_nxt = bnd_tiles[t % 2]
                ps = ppool.tile([128, NC20], f32, tag="ps")
                for c in range(NB):
                    cols = slice(NSEQ * c, NSEQ * (c + 1))
                    pc = (c - 1) % NB
                    nc.tensor.matmul(ps[:, cols], sbW_c[c], a_cur[:, cols],
                                     start=True, stop=False)
                    nc.tensor.matmul(ps[0:3, cols], sbWb_c[c],
                                     b_cur[0:32, NSEQ * pc:NSEQ * (pc + 1)],
                                     start=False, stop=True)
                nc.vector.tensor_tensor(out=a_nxt[:], in0=ps[:], in1=Eap,
                                        op=AO.mult)
                nc.vector.tensor_tensor(out=b_nxt[:], in0=ps[96:128, :],
                                        in1=Eap[96:128, :], op=AO.mult)
                if t % R == 0 or t == T - 1:
                    renorm(t, a_nxt)
                    if t != T - 1:
                        scale_steps.add(t + LAG)

            nc.sync.dma_start(out=ll_d[:], in_=llacc[:])
    _split_multi_waits(nc)
    return nc


_PROGRAM_CACHE = {}
LAST_EXEC_NS = None


def _get_program(T):
    if T not in _PROGRAM_CACHE:
        _PROGRAM_CACHE[T] = _build_program(T)
    return _PROGRAM_CACHE[T]


def kernel(inputs, init_kernel, transition_kernel, emission_kernel,
           A_w_idx, A_c_idx, B_w_idx, B_c_idx, I_idx, _T=None, _trace=False):
    global LAST_EXEC_NS
    inputs = np.asarray(inputs, dtype=np.float32)
    B, T = inputs.shape[0], inputs.shape[1]
    if _T is not None:
        T = _T
        inputs = inputs[:, :T]
    params = _host_params(np.asarray(init_kernel), np.asarray(transition_kernel),
                          np.asarray(emission_kernel))
    nc = _get_program(T)
    per_core = B // NCORES
    assert per_core == NSEQ
    in_maps = []
    for i in range(NCORES):
        shard = inputs[i * NSEQ:(i + 1) * NSEQ].reshape(NSEQ * T, EE)
        in_maps.append(dict(x=np.ascontiguousarray(shard), **params))
    results, exec_ns = _run_spmd_timed(nc, in_maps, NCORES,
                                        time_iters=(5 if _trace else 0))
    LAST_EXEC_NS = exec_ns
    out = np.concatenate([results[i]["ll"][0] for i in range(NCORES)])
    return out.astype(np.float32)


def _run_spmd_timed(nc, in_maps, n_cores, time_iters=0):
    """run_bass_via_pjrt clone that keeps the jitted executable and, when
    time_iters > 0, re-dispatches it with device-resident inputs to measure
    steady-state execution wall time (amortizing dispatch latency)."""
    import time as _time
    import jax
    from jax.sharding import Mesh, PartitionSpec
    from jax.experimental.shard_map import shard_map
    import concourse.mybir as _mybir

    bass2jax.install_neuronx_cc_hook()
    assert nc.dbg_addr is None or not nc.dbg_callbacks
    extra = {}
    if nc.dbg_addr is not None:
        extra[nc.dbg_addr.name] = np.zeros((1, 2), np.uint32)
        in_maps = [{**m, **extra} for m in in_maps]
    partition_name = nc.partition_id_tensor.name if nc.partition_id_tensor else None
    in_names, out_names, out_avals, zero_outs = [], [], [], []
    for alloc in nc.m.functions[0].allocations:
        if not isinstance(alloc, _mybir.MemoryLocationSet):
            continue
        name = alloc.memorylocations[0].name
        if alloc.kind == "ExternalInput":
            if name != partition_name:
                in_names.append(name)
        elif alloc.kind == "ExternalOutput":
            shape = tuple(alloc.tensor_shape)
            dtype = _mybir.dt.np(alloc.dtype)
            out_names.append(name)
            out_avals.append(jax.core.ShapedArray(shape, dtype))
            zero_outs.append(np.zeros(shape, dtype))
    n_params, n_outs = len(in_names), len(out_avals)
    all_in_names = in_names + out_names + ([partition_name] if partition_name else [])

    def _body(*args):
        operands = list(args)
        if partition_name is not None:
            operands.append(bass2jax.partition_id_tensor())
        outs = bass2jax._bass_exec_p.bind(
            *operands, out_avals=tuple(out_avals), in_names=tuple(all_in_names),
            out_names=tuple(out_names), lowering_input_output_aliases=(),
            sim_require_finite=True, sim_require_nnan=True, nc=nc)
        return tuple(outs)

    devices = jax.devices()[:n_cores]
    mesh = Mesh(np.asarray(devices), ("core",))
    in_specs = (PartitionSpec("core"),) * (n_params + n_outs)
    out_specs = (PartitionSpec("core"),) * len(out_names)
    donate = tuple(range(n_params, n_params + n_outs))
    fn = jax.jit(shard_map(_body, mesh=mesh, in_specs=in_specs,
                           out_specs=out_specs, check_rep=False),
                 donate_argnums=donate, keep_unused=True)
    concat_in = [np.concatenate([np.asarray(in_maps[c][nm]) for c in range(n_cores)],
                                axis=0) for nm in in_names]
    import jax.numpy as jnp
    from jax.sharding import NamedSharding
    shardings = [NamedSharding(mesh, PartitionSpec("core"))] * len(concat_in)
    dev_in = [jax.device_put(a, s) for a, s in zip(concat_in, shardings)]
    def zouts():
        return [np.concatenate([z] * n_cores, axis=0) for z in zero_outs]
    out_arrs = fn(*dev_in, *zouts())
    jax.block_until_ready(out_arrs)
    exec_ns = None
    if time_iters > 0:
        t0 = _time.time()
        last = None
        for _ in range(time_iters):
            last = fn(*dev_in, *zouts())
        jax.block_until_ready(last)
        exec_ns = int((_time.time() - t0) / time_iters * 1e9)
        out_arrs = last
    results = []
    for c in range(n_cores):
        m = {}
        for i, name in enumerate(out_names):
            full = np.asarray(out_arrs[i])
            per = full.shape[0] // n_cores
            m[name] = full[c * per:(c + 1) * per]
        results.append(m)
    return results, exec_ns


if __name__ == "__main__":
    # quick self-test at reduced T against numpy oracle
    import validate_numpy as V
    rng = np.random.default_rng(0)
    Tt = 256
    A_w, A_c, B_w, B_c, I_idx = V.make_indices()
    obs = rng.integers(0, 100, size=(32, Tt))
    x = np.eye(EE, dtype=np.float32)[obs]
    inp = dict(inputs=x, init_kernel=rng.normal(size=4).astype(np.float32),
               transition_kernel=rng.normal(size=A_w.shape[0]).astype(np.float32),
               emission_kernel=rng.normal(size=B_w.shape[0]).astype(np.float32),
               A_w_idx=A_w, A_c_idx=A_c, B_w_idx=B_w, B_c_idx=B_c, I_idx=I_idx)
    ref = V.ref_numpy(**inp)
    got = kernel(**inp)
    rel = np.abs(got - ref) / np.abs(ref)
    print("got[:4] :", got[:4])
    print("ref[:4] :", ref[:4])
    print("max rel err:", rel.max())
